# revision 14
# baseline (speedup 1.0000x reference)
"""NodeConv kernel for 8 Trainium2 NeuronCores.

Reference computes, for adj [B,1,N,N], node [B,nin,N], Wi/Wj [nout,nin]:
    x  = node[:, :, None, :] * adj          # [B,nin,N,N]
    yi = einsum('oc,bcij->boij', Wi, x)
    yj = einsum('oc,bcij->boij', Wj, x)
    out = I * yi + (1-I) * yj

Because adj[b,i,j] does not depend on the contraction channel c, the
contraction factors out:
    off-diag: out[b,o,i,j] = adj[b,i,j] * (Wj @ node[b])[o,j]
    diag:     out[b,o,j,j] = adj[b,j,j] * (Wi @ node[b])[o,j]

So per batch we need two tiny matmuls (u = Wj@node, v = Wi@node) and a
broadcast multiply out[o,i,j] = adj[i,j]*u[o,j] with a diagonal patch.

The output write is the memory roofline.  The device computes and stores
the output in FP16 (worst-case relative error ~1.5e-3, well inside the
2e-2 gate), which halves HBM store traffic to 8 MiB/core; the host casts
back to f32 while gathering.

Sharding: core c handles batch b=c//2, row half h=c%2 (128 rows). Odd
halves get their columns rolled by -128 on the host so the diagonal of
local row l sits at local column l on every core -> one SPMD program;
the host rolls the output back while gathering.

Per-core device program:
  - all small tensors fp16; u = Wj @ node (PE), u8 = fp16(u) replicated
    8x along free (DVE copy) so hot-loop TTs read dense step-1 fp16
  - v = Wi @ node[:, :128], dv16[o,l] = adj_diag[l] * v[o,l]
  - per 8-row chunk p: PE broadcasts the 8 adj rows to 128 partitions
    with K=16 one-hot selector matmuls.  The PE runs at 1.2 GHz on this
    platform and a K=16 matmul uses only 16 of its 128 rows, so pk is
    replicated at partition offsets 0/32/64/96 and chunk p issues its
    matmuls on row group 32*(p%4) via tile_position -- consecutive
    chunks' matmul streams execute concurrently in disjoint array rows
    (~3x on the broadcast wall).
  - multiply paths per chunk (split keeps every engine under the ~25us
    DMA floor):
      direct: DVE tensor_mul(out_fp16, psum_f32, u8)        (1x mode)
      staged: ScalarE casts psum->SBUF fp16, then DVE or GpSimd
              tensor_mul(out_fp16, stage_fp16, u8)          (DVE 2x)
    GpSimd patches the 8 diagonal elements via a stride-257 view.
  - stores go out in groups on the sync/scalar HWDGE rings.
"""

import os

import numpy as np

NCORES = 8
B, N, NIN, NOUT = 4, 256, 128, 128
RPC = 128          # rows per core
CH = 16            # chunks per core
RCH = 8            # rows per chunk
FREE = RCH * N     # 2048 free elems per chunk
NGRP = 4           # PE row groups used for broadcast matmul packing

# chunks whose multiply reads PSUM directly on DVE (1x); the rest are
# cast to fp16 by ScalarE first and multiplied on DVE (2x) or GpSimd.
_DIRECT = {
    int(x)
    for x in os.environ.get("NODECONV_DIRECT", "2,5,8,11,15").split(",")
    if x != ""
}
_GP_MULS = {
    int(x)
    for x in os.environ.get("NODECONV_GP_MULS", "").split(",")
    if x != ""
}
# engine for the 8-element diagonal patches: gpsimd | scalar | vector
_PATCH = os.environ.get("NODECONV_PATCH", "gpsimd")
# store group sizes (chunks per dma); small first/last groups shorten
# the pipeline fill and drain, big middle groups raise DMA efficiency.
_G = [
    int(x) for x in os.environ.get("NODECONV_G", "1,1,2,2,2,2,2,2,1,1").split(",")
]
assert sum(_G) == CH
OUT_BUFS = int(os.environ.get("NODECONV_OUT_BUFS", "8"))
STAGE_BUFS = int(os.environ.get("NODECONV_STAGE_BUFS", "6"))
# ring pattern for output stores: s=sync(HWDGE) a=scalar(HWDGE) g=gpsimd(SWDGE)
_RINGS = os.environ.get("NODECONV_RINGS", "sgsgsgsgss")

_cached = {}

last_results = None  # BassKernelResults of the most recent kernel() call


def _build_nc():
    key = (
        tuple(sorted(_DIRECT)),
        tuple(sorted(_GP_MULS)),
        _PATCH,
        tuple(_G),
        OUT_BUFS,
        STAGE_BUFS,
        _RINGS,
    )
    if key in _cached:
        return _cached[key]

    from contextlib import ExitStack

    import concourse.tile as tile
    from concourse import bacc, mybir

    f16 = mybir.dt.float16

    nc = bacc.Bacc(
        "TRN2", target_bir_lowering=False, debug=False, num_devices=NCORES
    )

    # pk: [16, 2*FREE] f16 — adj chunk rows in [:, :FREE], one-hot
    # selector blocks in [:, FREE:].  Loaded NGRP times at partition
    # offsets 32*q for the row-group matmul packing.
    pk = nc.dram_tensor("pk", [CH, 2 * FREE], f16, kind="ExternalInput").ap()
    # ckf: [128, 640] f16 — node_r | WiT | WjT | node_r[:, :128]*adj_diag
    ckf = nc.dram_tensor(
        "ckf", [NIN, N + 2 * NOUT + RPC], f16, kind="ExternalInput"
    ).ap()
    out = nc.dram_tensor("out", [NOUT, RPC * N], f16, kind="ExternalOutput").ap()

    ring_of = {"s": "sync", "a": "scalar", "g": "gpsimd"}
    patch_eng_name = {"gpsimd": "gpsimd", "scalar": "scalar", "vector": "vector"}[
        _PATCH
    ]

    with tile.TileContext(nc) as tc, ExitStack() as ctx:
        const = ctx.enter_context(tc.tile_pool(name="const", bufs=1))
        psum = ctx.enter_context(tc.tile_pool(name="psum", bufs=2, space="PSUM"))
        outp = ctx.enter_context(tc.tile_pool(name="outp", bufs=OUT_BUFS))
        stage = ctx.enter_context(tc.tile_pool(name="stage", bufs=STAGE_BUFS))

        # pk row-group 0 first: the chunk-0 broadcast is the critical path
        pk_sb = const.tile([32 * (NGRP - 1) + CH, 2 * FREE], f16)
        nc.sync.dma_start(out=pk_sb[0:CH, :], in_=pk)
        ckf_sb = const.tile([NIN, N + 2 * NOUT + RPC], f16)
        nc.scalar.dma_start(out=ckf_sb[:], in_=ckf)
        for q in range(1, NGRP):
            nc.gpsimd.dma_start(out=pk_sb[32 * q : 32 * q + CH, :], in_=pk)

        node_sb = ckf_sb[:, 0:N]
        wit_sb = ckf_sb[:, N : N + NOUT]
        wjt_sb = ckf_sb[:, N + NOUT : N + 2 * NOUT]
        nd_sb = ckf_sb[:, N + 2 * NOUT : N + 2 * NOUT + RPC]

        def chunk_mms(p):
            # the 4 PSUM-bank slices of one chunk go to 4 different PE row
            # groups (pk is replicated at partition 0/32/64/96), so the 4
            # matmuls stream concurrently in disjoint array rows and the
            # chunk's broadcast takes ~1 matmul-time of wall clock.
            ps_b = psum.tile([NOUT, FREE], mybir.dt.float32, tag="mm")
            for q in range(FREE // 512):
                sl = slice(512 * q, 512 * (q + 1))
                base = 32 * (q % NGRP)
                nc.tensor.matmul(
                    ps_b[:, sl],
                    lhsT=pk_sb[
                        base : base + CH, FREE + NOUT * p : FREE + NOUT * (p + 1)
                    ],
                    rhs=pk_sb[base : base + CH, sl],
                    start=True,
                    stop=True,
                    tile_position=(base, 0),
                )
            return ps_b

        # chunk 0's broadcast and its ScalarE cast first: the cast chain is
        # the steady-state limiter, so it starts before the u/dv setup.
        ps_b0 = chunk_mms(0)
        st0 = stage.tile([NOUT, FREE], f16, tag="st")
        nc.scalar.copy(st0[:], ps_b0[:])

        # u = Wj @ node_r -> [nout, N]; ScalarE casts to fp16, then one
        # 4x-mode DVE copy replicates 8x along the free dim so hot-loop
        # TTs read a dense step-1 fp16 operand.
        ps_u = psum.tile([NOUT, N], mybir.dt.float32, tag="mm")
        nc.tensor.matmul(ps_u[:], lhsT=wjt_sb, rhs=node_sb, start=True, stop=True)
        u16 = const.tile([NOUT, N], f16)
        nc.scalar.copy(u16[:], ps_u[:])
        u8 = const.tile([NOUT, FREE], f16)
        nc.vector.tensor_copy(
            u8[:].rearrange("p (k j) -> p k j", k=RCH),
            u16[:].unsqueeze(1).broadcast_to([NOUT, RCH, N]),
        )

        # dv = Wi @ (node_r * adj_diag)[:, :128]: the host pre-scales the
        # diagonal columns of node by adj's diagonal, so the diag patch
        # values come straight out of the conv matmul.
        ps_dv = psum.tile([NOUT, RPC], mybir.dt.float32, tag="mm")
        nc.tensor.matmul(ps_dv[:], lhsT=wit_sb, rhs=nd_sb, start=True, stop=True)
        dv16 = const.tile([NOUT, RPC], f16)
        nc.scalar.copy(dv16[:], ps_dv[:])

        starts = []
        p = 0
        for gsz in _G:
            starts.append(p)
            p += gsz
        order = list(range(len(_G)))
        if len(_G) >= 2 and _G[-1] == 1 and _G[-2] == 1:
            # process the final chunk's group early so the kernel tail ends
            # on a cheap staged TT instead of a long direct one
            order[-2], order[-1] = order[-1], order[-2]
        for oi, gi in enumerate(order):
            gsz = _G[gi]
            p0 = starts[gi]
            o_sb = outp.tile([NOUT, gsz * FREE], f16, tag="osb")
            for g in range(gsz):
                p = p0 + g
                ps_b = ps_b0 if p == 0 else chunk_mms(p)
                o_view = o_sb[:, g * FREE : (g + 1) * FREE]
                if p in _DIRECT:
                    nc.vector.tensor_mul(o_view, ps_b[:], u8[:])
                else:
                    if p == 0:
                        st_sb = st0
                    else:
                        st_sb = stage.tile([NOUT, FREE], f16, tag="st")
                        nc.scalar.copy(st_sb[:], ps_b[:])
                    eng = nc.gpsimd if p in _GP_MULS else nc.vector
                    eng.tensor_mul(o_view, st_sb[:], u8[:])
                # diagonal of local row l=8p+k sits at free offset 8p + k*257
                patch_eng = getattr(nc, patch_eng_name)
                patch_copy = (
                    patch_eng.copy
                    if patch_eng_name == "scalar"
                    else patch_eng.tensor_copy
                )
                patch_copy(
                    o_sb[
                        :,
                        g * FREE + RCH * p : g * FREE
                        + RCH * p
                        + (RCH - 1) * (N + 1)
                        + 1 : N + 1,
                    ],
                    dv16[:, RCH * p : RCH * (p + 1)],
                )
            pe = p0 + gsz
            if oi == len(order) - 1:
                half = gsz * FREE // 2
                nc.sync.dma_start(
                    out=out[:, FREE * p0 : FREE * p0 + half], in_=o_sb[:, 0:half]
                )
                nc.scalar.dma_start(
                    out=out[:, FREE * p0 + half : FREE * pe],
                    in_=o_sb[:, half : gsz * FREE],
                )
            else:
                eng = getattr(nc, ring_of[_RINGS[gi % len(_RINGS)]])
                eng.dma_start(out=out[:, FREE * p0 : FREE * pe], in_=o_sb[:])

    nc.compile()
    _cached[key] = nc
    return nc


def _in_maps(adj, node, Wi, Wj):
    sel = np.zeros((CH, CH * NOUT), np.float16)
    for p in range(CH):
        sel[p, NOUT * p : NOUT * (p + 1)] = 1.0
    ckf = np.empty((NIN, N + 2 * NOUT + RPC), np.float16)
    ckf[:, N : N + NOUT] = Wi.T.astype(np.float16)
    ckf[:, N + NOUT : N + 2 * NOUT] = Wj.T.astype(np.float16)
    maps = []
    for c in range(NCORES):
        b, h = divmod(c, 2)
        r0 = RPC * h
        a = adj[b, 0, r0 : r0 + RPC, :]
        diag = a[np.arange(RPC), r0 + np.arange(RPC)]
        if h:
            ar = np.roll(a, -r0, axis=1)
            noder = np.roll(node[b], -r0, axis=1)
        else:
            ar = a
            noder = node[b]
        pk = np.empty((CH, 2 * FREE), np.float16)
        pk[:, 0:FREE] = ar.reshape(CH, FREE).astype(np.float16)
        pk[:, FREE:] = sel
        m_ckf = ckf.copy()
        m_ckf[:, 0:N] = noder.astype(np.float16)
        m_ckf[:, N + 2 * NOUT :] = (noder[:, 0:RPC] * diag[None, :]).astype(
            np.float16
        )
        maps.append({"pk": pk, "ckf": m_ckf})
    return maps


def kernel(**inputs):
    global last_results
    adj = np.asarray(inputs["adj"], dtype=np.float32)
    node = np.asarray(inputs["node"], dtype=np.float32)
    Wi = np.asarray(inputs["Wi"], dtype=np.float32)
    Wj = np.asarray(inputs["Wj"], dtype=np.float32)

    from concourse.bass_utils import run_bass_kernel_spmd

    nc = _build_nc()
    res = run_bass_kernel_spmd(nc, _in_maps(adj, node, Wi, Wj), list(range(NCORES)))
    last_results = res

    out = np.empty((B, NOUT, N, N), np.float32)
    for c in range(NCORES):
        b, h = divmod(c, 2)
        co = (
            np.asarray(res.results[c]["out"])
            .astype(np.float32)
            .reshape(NOUT, RPC, N)
        )
        if h:
            co = np.roll(co, RPC * h, axis=2)
        out[b, :, RPC * h : RPC * (h + 1), :] = co
    return out
